# revision 25
# baseline (speedup 1.0000x reference)
"""Expert-parallel MoE feed-forward for Trainium2 (8 NeuronCores).

Strategy:
  - Host: gate + top-2 routing (0.02% of FLOPs), builds per-expert token
    index lists.  Expert e is owned by core e.
  - Device (same SPMD program on all 8 cores): indirect-DMA gather of the
    expert's tokens, FFN  y = relu(x@W1+b1)@W2+b2  in bf16 (full PE rate,
    fp32 PSUM accumulation), scale by combine weight, write y^T [D, C].
  - Host: scatter-add compact results into the [B,S,D] output.

Per-core pipeline (single pass over TUSE <= C token columns):
  W1 resident in SBUF (8 x 1MB DMAs, 8KB lines).
  gather tile g: xg[t, d] = x[tok(g*128+t), d]            (indirect DMA)
  PE-transpose:  xT_c[p, k*384 + g*128 + t] = xg[t, k*128+p]
  mm1: acc[f, tok] = sum_k W1[k-chunk, j-tile].T @ xT[k]  (fp32 PSUM)
       h[f, tok] = relu(acc + b1)                         (bf16, ACT)
  mm2: acc2[d, tok] = sum_j W2[j, d-tile].T @ h[j, tok]   (W2 stationary,
       streamed once via host-prepacked [P, KD*NJ*P] layout)
  y^T[d, tok] = (acc2 + b2) * wc[tok]                     (DVE), DMA out.
"""

import numpy as np

B, S, D, F, E = 2, 2048, 1024, 4096, 8
T = B * S                      # 4096 tokens
K_TOP = 2
P = 128
C = 1152                       # per-expert token capacity (9 * 128)
KD = D // P                    # 8 contraction tiles for mm1
NJ = F // P                    # 32 f-tiles
GT = C // P                    # 9 gather tiles
GPC = 3                        # gather tiles per chunk
CW = GPC * P                   # chunk width (384 token columns)

_CACHE = {}


def _chunks(tuse):
    out = []
    for t0 in range(0, tuse, CW):
        out.append((t0, min(CW, tuse - t0)))
    return out


def _build_program(tuse, loop_n=1, skip_mm1=False, skip_mm2=False,
                   hoist_gather=False, hoist_w1=False,
                   b1mm=4, b2mm=4, bw2=3, il1=2):
    import concourse.bass as bass
    import concourse.mybir as mybir
    import concourse.tile as tile
    from concourse import bacc
    from contextlib import ExitStack

    f32 = mybir.dt.float32
    bf16 = mybir.dt.bfloat16
    i32 = mybir.dt.int32

    nc = bacc.Bacc("TRN2", target_bir_lowering=False, debug=False)

    # x pre-gathered+pre-transposed on host, chunk-contiguous:
    # xt[p, ci*KD*CW + k*CW + tl] = x[tok(ci*CW+tl), k*P + p]
    xt_d = nc.dram_tensor("xt", [P, (GT // GPC) * KD * CW], bf16,
                          kind="ExternalInput").ap()
    # W1 prepacked on host: w1p[p, (j*KD + k)*P + c] = W1[k*P + p, j*P + c]
    w1_d = nc.dram_tensor("W1p", [P, NJ * KD * P], bf16,
                          kind="ExternalInput").ap()
    # W2 prepacked on host: w2p[p, (d*NJ + j)*P + c] = W2[j*P + p, d*P + c]
    w2_d = nc.dram_tensor("W2p", [P, KD * NJ * P], bf16,
                          kind="ExternalInput").ap()
    # combine weights replicated across partitions: wcr[p, t] = wc[t]
    wcr_d = nc.dram_tensor("wcr", [P, C], f32, kind="ExternalInput").ap()
    b1_d = nc.dram_tensor("b1t", [P, NJ], f32, kind="ExternalInput").ap()
    b2_d = nc.dram_tensor("b2c", [P, KD], f32, kind="ExternalInput").ap()
    y_d = nc.dram_tensor("yout", [D, C], f32, kind="ExternalOutput").ap()

    chunks = _chunks(tuse)
    n_gt = (tuse + P - 1) // P

    with tile.TileContext(nc) as tc, ExitStack() as ctx:
        sb = ctx.enter_context(tc.tile_pool(name="sb", bufs=1))
        ps = ctx.enter_context(tc.tile_pool(name="ps", bufs=1, space="PSUM"))

        wcr_t = sb.tile([P, C], f32, tag="wcr")
        b1_t = sb.tile([P, NJ], f32, tag="b1")
        b2_t = sb.tile([P, KD], f32, tag="b2")
        nc.sync.dma_start(wcr_t[:], wcr_d[:])
        nc.sync.dma_start(b1_t[:], b1_d[:])
        nc.sync.dma_start(b2_t[:], b2_d[:])

        h = sb.tile([P, NJ * C], bf16, tag="h", name="h")
        if skip_mm1:
            nc.vector.memset(h[:], 0.0)

        def emit_w1(js=range(NJ)):
            w1t = []
            for j in js:
                t = sb.tile([P, KD * P], bf16, tag="w1", bufs=NJ + 1,
                            name=f"w1_{j}")
                nc.sync.dma_start(t[:], w1_d[:, j * KD * P:(j + 1) * KD * P])
                w1t.append(t)
            return w1t

        def emit_gather(ci):
            # host pre-gathered/pre-transposed: one contiguous DMA per chunk
            xT = sb.tile([P, KD * CW], bf16, tag="xT", bufs=3,
                         name=f"xT_{ci}")
            nc.sync.dma_start(
                xT[:], xt_d[:, ci * KD * CW:(ci + 1) * KD * CW])
            return xT

        if hoist_w1:
            w1t = emit_w1()
        if hoist_gather:
            xTs = {ci: emit_gather(ci) for ci in range(len(chunks))}

        loop_cm = tc.For_i(0, loop_n, 1) if loop_n > 1 else None
        if loop_cm is not None:
            loop_cm.__enter__()

        # Emit each chunk's gather+transpose BEFORE a slice of the W1 loads:
        # the SP queue is in-order, so transposes must not sit behind all
        # 32 W1 DMAs or the PE idles ~24us at every iteration start.
        if not hoist_gather:
            xTs = {}
            w1_split = [range(0, 4), range(4, 16), range(16, NJ)]
            for ci in range(len(chunks)):
                xTs[ci] = emit_gather(ci)
                if not hoist_w1 and ci < len(w1_split):
                    js = w1_split[ci] if ci < len(chunks) - 1 else \
                        range(w1_split[ci].start, NJ)
                    w1t_part = emit_w1(js)
                    if ci == 0:
                        w1t = w1t_part
                    else:
                        w1t += w1t_part
        elif not hoist_w1:
            w1t = emit_w1()

        # --- per chunk: mm1 + relu -> h ---
        # j-groups emitted in ping-pong pairs: alternating PSUM banks every
        # matmul hides the per-group start/stop bubble under the sibling
        # group's stream
        for ci, (t0, ns) in enumerate(chunks):
            xT = xTs[ci]
            if skip_mm1:
                continue
            for j0 in range(0, NJ, il1):
                width = min(il1, NJ - j0)
                accs = [ps.tile([P, ns], f32, tag="mm1", bufs=b1mm,
                                name=f"p1_{ci}_{j0 + u}")
                        for u in range(width)]
                for k in range(KD):
                    for u in range(width):
                        nc.tensor.matmul(
                            accs[u][:],
                            lhsT=w1t[j0 + u][:, k * P:(k + 1) * P],
                            rhs=xT[:, k * CW: k * CW + ns],
                            start=(k == 0), stop=(k == KD - 1))
                for u in range(width):
                    j = j0 + u
                    nc.scalar.activation(
                        h[:, j * C + t0: j * C + t0 + ns], accs[u][:],
                        mybir.ActivationFunctionType.Relu,
                        bias=b1_t[:, j:j + 1])

        # --- mm2: y^T[d, tok] = sum_j W2[j, d].T @ h[j, tok] ---
        for d in range(KD if not skip_mm2 else 0):
            w2t = sb.tile([P, NJ * P], bf16, tag="w2", bufs=bw2,
                          name=f"w2_{d}")
            nc.sync.dma_start(w2t[:], w2_d[:, d * NJ * P:(d + 1) * NJ * P])
            for ci, (t0, ns) in enumerate(chunks):
                acc2 = ps.tile([P, ns], f32, tag="mm2", bufs=b2mm,
                               name=f"p2_{d}_{ci}")
                for j in range(NJ):
                    nc.tensor.matmul(
                        acc2[:],
                        lhsT=w2t[:, j * P:(j + 1) * P],
                        rhs=h[:, j * C + t0: j * C + t0 + ns],
                        start=(j == 0), stop=(j == NJ - 1))
                ot = sb.tile([P, ns], f32, tag="ot", bufs=2,
                             name=f"ot_{d}_{ci}")
                nc.vector.tensor_scalar_add(ot[:], acc2[:], b2_t[:, d:d + 1])
                nc.vector.tensor_tensor(
                    out=ot[:], in0=ot[:], in1=wcr_t[:, t0:t0 + ns],
                    op=mybir.AluOpType.mult)
                # y-stores go on the ACT HWDGE queue so they never block
                # next-iteration weight prefetch on the SP queue
                nc.scalar.dma_start(
                    y_d[d * P:(d + 1) * P, t0:t0 + ns], ot[:])

        if loop_cm is not None:
            loop_cm.__exit__(None, None, None)

    nc.compile()
    return nc


def _route(x2, Wg, bg):
    """Host-side top-2 routing in float64 (stable ordering)."""
    gate = x2.astype(np.float64) @ np.asarray(Wg, np.float64) + np.asarray(bg, np.float64)
    part = np.argpartition(-gate, K_TOP - 1, axis=1)[:, :K_TOP]      # [T, 2]
    rows = np.arange(T)[:, None]
    sc = gate[rows, part]                                            # [T, 2]
    sc = sc - sc.max(axis=1, keepdims=True)
    e_sc = np.exp(sc)
    probs = e_sc / e_sc.sum(axis=1, keepdims=True)                   # [T, 2]
    idx_e, w_e, n_e = [], [], []
    for e in range(E):
        mask = part == e                                             # [T, 2]
        tok = np.nonzero(mask.any(axis=1))[0]
        pr = probs[mask]                                             # aligned with tok
        n = len(tok)
        pad = C - n
        if pad < 0:
            return None                                              # capacity overflow
        idx_e.append(np.concatenate([tok, np.zeros(pad, np.int64)]).astype(np.int32))
        w_e.append(np.concatenate([pr, np.zeros(pad)]).astype(np.float32))
        n_e.append(n)
    return idx_e, w_e, n_e


def _prepack_w2(W2e_bf16):
    """[F, D] -> [P, KD*NJ*P]: w2p[p, (d*NJ+j)*P + c] = W2[j*P+p, d*P+c]."""
    w = W2e_bf16.reshape(NJ, P, KD, P)          # (j, p, d, c)
    w = w.transpose(1, 2, 0, 3)                 # (p, d, j, c)
    return np.ascontiguousarray(w.reshape(P, KD * NJ * P))


def _prepack_xt(xs_bf16):
    """[C, D] gathered tokens -> [P, NCH*KD*CW] chunk-contiguous transposed:
    xt[p, ci*KD*CW + k*CW + tl] = xs[ci*CW + tl, k*P + p]."""
    nch = C // CW
    a = xs_bf16.reshape(nch, CW, KD, P)         # (ci, tl, k, p)
    a = a.transpose(3, 0, 2, 1)                 # (p, ci, k, tl)
    return np.ascontiguousarray(a.reshape(P, nch * KD * CW))


def _prepack_w1(W1e_bf16):
    """[D, F] -> [P, NJ*KD*P]: w1p[p, (j*KD+k)*P + c] = W1[k*P+p, j*P+c]."""
    w = W1e_bf16.reshape(KD, P, NJ, P)          # (k, p, j, c)
    w = w.transpose(1, 2, 0, 3)                 # (p, j, k, c)
    return np.ascontiguousarray(w.reshape(P, NJ * KD * P))


def kernel(x, W1, b1, W2, b2, Wg, bg, num_experts_per_token):
    import ml_dtypes
    from concourse.bass_utils import run_bass_kernel_spmd

    bf16 = ml_dtypes.bfloat16
    x2 = np.ascontiguousarray(np.asarray(x, np.float32).reshape(T, D))
    W1 = np.asarray(W1, np.float32)
    b1 = np.asarray(b1, np.float32)
    W2 = np.asarray(W2, np.float32)
    b2 = np.asarray(b2, np.float32)

    routing = _route(x2, Wg, bg)
    if routing is None or int(num_experts_per_token) != K_TOP:
        # capacity overflow or unexpected top-k: correct slow path
        gate = x2.astype(np.float64) @ np.asarray(Wg, np.float64) + np.asarray(bg, np.float64)
        k = int(num_experts_per_token)
        part = np.argsort(-gate, axis=1)[:, :k]
        sc = gate[np.arange(T)[:, None], part]
        sc = sc - sc.max(axis=1, keepdims=True)
        pr = np.exp(sc); pr /= pr.sum(axis=1, keepdims=True)
        out = np.zeros((T, D), np.float32)
        for e in range(E):
            mask = part == e
            tok = np.nonzero(mask.any(axis=1))[0]
            w = pr[mask].astype(np.float32)
            hcur = np.maximum(x2[tok] @ W1[e] + b1[e], 0.0)
            out[tok] += w[:, None] * (hcur @ W2[e] + b2[e])
        return out.reshape(B, S, D)

    idx_e, w_e, n_e = routing
    tuse = min(C, ((max(n_e) + 3) // 4) * 4)

    key = ("nc", tuse)
    if key not in _CACHE:
        _CACHE[key] = _build_program(tuse)
    nc = _CACHE[key]

    x_bf = x2.astype(bf16)
    in_maps = []
    for e in range(E):
        in_maps.append({
            "xt": _prepack_xt(x_bf[idx_e[e]]),
            "W1p": _prepack_w1(W1[e].astype(bf16)),
            "W2p": _prepack_w2(W2[e].astype(bf16)),
            "wcr": np.ascontiguousarray(
                np.broadcast_to(w_e[e], (P, C))).astype(np.float32),
            "b1t": np.ascontiguousarray(b1[e].reshape(NJ, P).T),
            "b2c": np.ascontiguousarray(b2[e].reshape(KD, P).T),
        })

    res = run_bass_kernel_spmd(nc, in_maps, list(range(E)))

    out = np.zeros((T, D), np.float32)
    for e in range(E):
        n = n_e[e]
        out[idx_e[e][:n]] += res.results[e]["yout"][:, :n].T
    return out.reshape(B, S, D)
